# revision 1
# baseline (speedup 1.0000x reference)
"""CatLayer Trainium2 kernel (pure fp32).

Math: out[i,j,b,:] = W @ leaky_relu(concat(x[i,b,:], x[j,b,:])) + bias
Since leaky_relu is elementwise over the concat:
    y  = leaky_relu(x)                    # (l, b, d)
    A  = y @ W[:, :d].T + bias            # (l, b, d)   "xi half"
    B  = y @ W[:, d:].T                   # (l, b, d)   "xj half"
    out[i,j,b,:] = A[i,b,:] + B[j,b,:]

Sharding: i-rows of the (l x l) pair grid over 8 cores (12 rows each).
Every core computes B for all j from full x; A only for its own i rows
(supplied per-core as the packed xiT input).

Inputs are packed host-side into SBUF layout so each is a fully
contiguous DMA (partition dim leading):
    xT   (128, KT*T):   xT[p, k*T + t]  = x[t, 128k+p]
    xiT  (128, KT*TI):  xiT[p, k*TI+ti] = x_own[ti, 128k+p]
    W_in (128, 8*D):    W_in[p, g*D+c]  = W.T[128g+p, c]   (g<4: W1, g>=4: W2)
    bias (1, D)
    out  (12*l*b, d)

Engines:
    ACT: Prelu(alpha=0.1) + B-path PSUM->SBUF drains
    PE : A/B matmuls (fp32), one-hot E-matmul for the 16->128 partition
         broadcast of A[i]
    DVE: tensor_add for all output tiles + A-path PSUM drains
    DMA: big contiguous transfers; out stores are one per (i, j-group)

The j-tiles are grouped with a small first group so the first out-DMA can
start as soon as the first B tile is drained.
"""

import numpy as np
from contextlib import ExitStack

import concourse.bacc as bacc
import concourse.mybir as mybir
from concourse import tile
from concourse.bass_utils import run_bass_kernel_spmd

F32 = mybir.dt.float32
AF = mybir.ActivationFunctionType

L, Bdim, D = 96, 16, 512
NCORES = 8
LPC = L // NCORES          # 12 i-rows per core
T = L * Bdim               # 1536 (j,b) rows
NT = T // 128              # 12 j-tiles
KT = D // 128              # 4 k-tiles
TI = LPC * Bdim            # 192 own (i,b) rows
NA = TI // 32              # 6 A-row groups of 32
NEG_SLOPE = 0.1


def build_nc(repeats: int = 1, group_sizes=(1, 1, 1, 1, 1, 2, 2, 3), gps_groups=()):
    """Build the per-core Bass program (identical on all cores)."""
    assert sum(group_sizes) == NT
    g_off = [0]
    for g in group_sizes:
        g_off.append(g_off[-1] + g)

    nc = bacc.Bacc("TRN2", target_bir_lowering=False, debug=False)

    xT = nc.dram_tensor("xT", (128, KT * T), F32, kind="ExternalInput")
    xiT = nc.dram_tensor("xiT", (128, KT * TI), F32, kind="ExternalInput")
    w_in = nc.dram_tensor("w_in", (128, 2 * KT * D), F32, kind="ExternalInput")
    bias = nc.dram_tensor("bias", (1, D), F32, kind="ExternalInput")
    out = nc.dram_tensor("out", (LPC * T, D), F32, kind="ExternalOutput")

    # One-hot E for the 16->128 partition broadcast of A rows, replicated
    # with period 32 down all 128 rows so any legal 32-aligned window has
    # identical content: ec[par][g, p] == 1 iff g % 32 == 16*par + p % 16
    ec_np = np.zeros((2, 128, 128), np.float32)
    for par in range(2):
        for g in range(128):
            for p in range(128):
                if g % 32 == 16 * par + (p % 16):
                    ec_np[par, g, p] = 1.0
    ec_dram = nc.inline_tensor(ec_np, "Ec")

    with tile.TileContext(nc) as tc, ExitStack() as ctx:
        persist = ctx.enter_context(tc.tile_pool(name="persist", bufs=1))
        stage = ctx.enter_context(tc.tile_pool(name="stage", bufs=2))
        psum = ctx.enter_context(tc.tile_pool(name="psum", bufs=6, space="PSUM"))
        outp = ctx.enter_context(tc.tile_pool(name="outp", bufs=3))

        # ---- small constants
        bias_sb = persist.tile([1, D], F32, tag="bias", name="bias_sb")
        nc.scalar.dma_start(bias_sb[:], bias[:])
        ones_sb = persist.tile([1, 128], F32, tag="ones", name="ones_sb")
        nc.vector.memset(ones_sb[:], 1.0)
        ec_all = persist.tile([128, 256], F32, tag="ec", name="ec_all")
        nc.scalar.dma_start(
            ec_all[:].rearrange("g (a p) -> g a p", a=2),
            ec_dram.ap().rearrange("a g p -> g a p"),
        )
        ec_sb = [ec_all[:, :128], ec_all[:, 128:]]
        w_sb = persist.tile([128, 2 * KT * D], F32, tag="w", name="w_sb")

        def w1s(k):
            return w_sb[:, k * D : (k + 1) * D]

        def w2s(k):
            return w_sb[:, (KT + k) * D : (KT + k + 1) * D]

        # ---- PE warm-up: HAM runs the PE at half clock until it has seen
        # ~3.4us of activity. Issue dummy matmuls (ones x ones) that depend
        # only on the memset so the array is at full clock when real matmuls
        # arrive with the first inputs.
        warm_ps = psum.tile([128, 64], F32, tag="eps", bufs=2, name="warm_ps")
        for _ in range(16):
            nc.tensor.matmul(
                warm_ps[:], ones_sb[:1, :], ones_sb[:1, :64],
                start=True, stop=True,
            )

        for rep in range(repeats):
            # ---- input DMAs: xiT (A path, small) first, then W1, then the
            # x slices with the W2 half interleaved. Each is contiguous.
            xi_st = stage.tile(
                [128, KT * TI], F32, tag="xi_st", bufs=1, name=f"xi_st_{rep}"
            )
            nc.sync.dma_start(xi_st[:], xiT[:])
            if rep == 0:
                nc.sync.dma_start(w_sb[:, : KT * D], w_in[:, : KT * D])
                nc.sync.dma_start(w_sb[:, KT * D :], w_in[:, KT * D :])
            x_st = stage.tile(
                [128, KT * T], F32, tag="x_st", bufs=1, name=f"x_st_{rep}"
            )
            # Every k slice is split at column 512: the first four B j-tiles
            # read only columns [0, 512) of each slice, so loading the
            # "a" halves first lets the first out-DMAs enter the sync FIFO
            # before the "b" halves (emitted after the first add group) --
            # otherwise the in-order HWDGE FIFO delays the first store
            # until the whole input stream has drained.
            XA = 512
            for k in range(KT):
                nc.sync.dma_start(
                    x_st[:, k * T : k * T + XA], xT[:, k * T : k * T + XA]
                )

            # ---- leaky relu, sliced per k-tile so B matmuls start per-slice
            yiT = persist.tile([128, KT * TI], F32, tag="yiT", name=f"yiT_{rep}")
            for k in range(KT):
                nc.scalar.activation(
                    yiT[:, k * TI : (k + 1) * TI],
                    xi_st[:, k * TI : (k + 1) * TI],
                    AF.Prelu,
                    alpha=NEG_SLOPE,
                )
            yT = persist.tile([128, KT * T], F32, tag="yT", name=f"yT_{rep}")
            for k in range(KT):
                nc.scalar.activation(
                    yT[:, k * T : k * T + XA],
                    x_st[:, k * T : k * T + XA],
                    AF.Prelu,
                    alpha=NEG_SLOPE,
                )

            def emit_xb():
                # the deferred b-halves: columns [512, T) of every k slice
                for k in range(KT):
                    nc.sync.dma_start(
                        x_st[:, k * T + XA : (k + 1) * T],
                        xT[:, k * T + XA : (k + 1) * T],
                    )
                for k in range(KT):
                    nc.scalar.activation(
                        yT[:, k * T + XA : (k + 1) * T],
                        x_st[:, k * T + XA : (k + 1) * T],
                        AF.Prelu,
                        alpha=NEG_SLOPE,
                    )

            # ---- A = leaky_relu(xi) @ W1.T + bias in three M-groups
            # (128, 32, 64 rows). The 32-row group re-covers rows 96..127 so
            # every E-matmul window can start at a legal base partition
            # (matmul operands must share base partition in {0, 32, 64}).
            a_parts = {}   # w -> (tile, offset)

            def emit_a(tag, rows, col0, windows):
                aps = psum.tile(
                    [rows, D], F32, tag="ps32", bufs=2,
                    padded_shape=[128, D], name=f"aps_{rep}_{tag}"
                )
                for k in range(KT):
                    nc.tensor.matmul(
                        aps[:],
                        yiT[:, k * TI + col0 : k * TI + col0 + rows],
                        w1s(k),
                        start=(k == 0),
                        stop=False,
                    )
                nc.tensor.matmul(
                    aps[:], ones_sb[:1, :rows], bias_sb[:1, :],
                    start=False, stop=True,
                )
                aw = persist.tile(
                    [rows, D], F32, tag=f"a_{tag}", name=f"a_{rep}_{tag}"
                )
                nc.vector.tensor_copy(aw[:], aps[:])
                for w, off in windows:
                    a_parts[w] = (aw, off)

            # Emission order = engine program order (engines run in-order),
            # so everything is emitted in expected-readiness order: a
            # "frontier" schedule where each producer is followed by the adds
            # it unlocks.
            out_v = out.rearrange("(i j p) c -> i p j c", i=LPC, p=128)
            abc = persist.tile([128, LPC * D], F32, tag="abc", name=f"abc_{rep}")
            n_grp = len(group_sizes)
            b_grp = [None] * n_grp

            def emit_bgroup(g):
                gsz = group_sizes[g]
                bg = persist.tile(
                    [128, gsz * D], F32, tag=f"b_grp{g}", name=f"b_grp{g}_{rep}"
                )
                for q in range(gsz):
                    jt = g_off[g] + q
                    bps = psum.tile(
                        [128, D], F32, tag="ps", bufs=4, name=f"bps_{rep}_{jt}"
                    )
                    for k in range(KT):
                        nc.tensor.matmul(
                            bps[:],
                            yT[:, k * T + 128 * jt : k * T + 128 * (jt + 1)],
                            w2s(k),
                            start=(k == 0),
                            stop=(k == KT - 1),
                        )
                    nc.scalar.activation(bg[:, q * D : (q + 1) * D], bps[:], AF.Copy)
                b_grp[g] = bg

            def emit_abc(il):
                w, par = divmod(il, 2)
                src, off = a_parts[w]
                eps = psum.tile(
                    [128, D], F32, tag="eps", bufs=2, name=f"eps_{rep}_{il}"
                )
                nc.tensor.matmul(
                    eps[:],
                    ec_sb[par][off : off + 32],
                    src[off : off + 32, :],
                    start=True,
                    stop=True,
                )
                # early slices drain on DVE (ACT is stuck behind the relus in
                # its in-order stream); later ones go to ACT which has slack.
                if il < 4:
                    nc.vector.tensor_copy(abc[:, il * D : (il + 1) * D], eps[:])
                else:
                    nc.scalar.activation(
                        abc[:, il * D : (il + 1) * D], eps[:], AF.Copy
                    )

            def emit_add(il, g):
                gsz = group_sizes[g]
                use_gps = False
                eng = nc.gpsimd if use_gps else nc.vector
                pool_tag = "og" if use_gps else "ot"
                ot = outp.tile(
                    [128, gsz * D], F32, tag=pool_tag, bufs=8,
                    name=f"ot_{rep}_{il}_{g}"
                )
                a_slice = abc[:, il * D : (il + 1) * D]
                eng.tensor_add(
                    ot[:].rearrange("p (j c) -> p j c", c=D),
                    b_grp[g][:].rearrange("p (j c) -> p j c", c=D),
                    a_slice.unsqueeze(1).broadcast_to((128, gsz, D)),
                )
                nc.sync.dma_start(
                    out_v[il, :, g_off[g] : g_off[g + 1], :],
                    ot[:].rearrange("p (j c) -> p j c", c=D),
                )

            ready_il = []
            ready_g = []

            def unlock_il(*ils):
                for il in ils:
                    emit_abc(il)
                for il in ils:
                    ready_il.append(il)
                    for g in ready_g:
                        emit_add(il, g)

            def unlock_g(g):
                emit_bgroup(g)
                ready_g.append(g)
                for il in ready_il:
                    emit_add(il, g)

            # windows: w0..2 live in the 128-row A group at offsets 0/32/64,
            # w3 in its own 32-row group, w4..5 in the 64-row group.
            a_specs = {
                0: ("g0", 128, 0, [(0, 0), (1, 32), (2, 64)]),
                3: ("g0b", 32, 96, [(3, 0)]),
                4: ("g1", 64, 128, [(4, 0), (5, 32)]),
            }
            pairs = [(2 * p, 2 * p + 1) for p in range(LPC // 2)]
            gi = 0
            xb_done = False
            for pi, pair in enumerate(pairs):
                if pi in a_specs:
                    emit_a(*a_specs[pi])
                unlock_il(*pair)
                while gi < n_grp and (gi + 1) * len(pairs) <= (pi + 1) * n_grp:
                    unlock_g(gi)
                    gi += 1
                    if not xb_done:
                        emit_xb()
                        xb_done = True
            while gi < n_grp:
                unlock_g(gi)
                gi += 1

    nc.compile()
    return nc


def _pack_kt(arr_t, nfree):
    """(D, nfree) k-major -> (128, KT*nfree) partition-packed SBUF layout."""
    return np.ascontiguousarray(
        arr_t.reshape(KT, 128, nfree).transpose(1, 0, 2).reshape(128, KT * nfree)
    )


def make_in_maps(x, W, bias):
    x = np.asarray(x, np.float32)
    W = np.asarray(W, np.float32)
    bias = np.asarray(bias, np.float32)
    xT = _pack_kt(np.ascontiguousarray(x.reshape(T, D).T), T)
    w_all = np.ascontiguousarray(
        np.ascontiguousarray(W.T)
        .reshape(2 * KT, 128, D)
        .transpose(1, 0, 2)
        .reshape(128, 2 * KT * D)
    )
    b2 = np.ascontiguousarray(bias.reshape(1, D))
    maps = []
    for r in range(NCORES):
        xi = _pack_kt(
            np.ascontiguousarray(x[r * LPC : (r + 1) * LPC].reshape(TI, D).T), TI
        )
        maps.append({"xT": xT, "xiT": xi, "w_in": w_all, "bias": b2})
    return maps


_NC_CACHE = {}


def get_nc(repeats=1, group_sizes=(1, 1, 1, 1, 1, 2, 2, 3), gps_groups=()):
    key = (repeats, tuple(group_sizes), tuple(gps_groups))
    if key not in _NC_CACHE:
        _NC_CACHE[key] = build_nc(repeats=repeats, group_sizes=group_sizes, gps_groups=gps_groups)
    return _NC_CACHE[key]


def kernel(x, W, bias, group_sizes=(1, 1, 1, 1, 1, 2, 2, 3), gps_groups=()):
    nc = get_nc(1, group_sizes, gps_groups)
    maps = make_in_maps(x, W, bias)
    res = run_bass_kernel_spmd(nc, maps, list(range(NCORES)))
    outs = [res.results[r]["out"] for r in range(NCORES)]
    return np.concatenate(outs, axis=0).reshape(L * L, Bdim, D)



# revision 2
# speedup vs baseline: 1.0394x; 1.0394x over previous
"""CatLayer Trainium2 kernel (fp16 inputs, fp32 accumulate/output).

Math: out[i,j,b,:] = W @ leaky_relu(concat(x[i,b,:], x[j,b,:])) + bias
Since leaky_relu is elementwise over the concat:
    y  = leaky_relu(x)                    # (l, b, d)
    A  = y @ W[:, :d].T + bias            # (l, b, d)   "xi half"
    B  = y @ W[:, d:].T                   # (l, b, d)   "xj half"
    out[i,j,b,:] = A[i,b,:] + B[j,b,:]

Sharding: i-rows of the (l x l) pair grid over 8 cores (12 rows each).
Every core computes B for all j from full x; A only for its own i rows.

The kernel is DMA-bound: the fp32 output (37.75 MB/core) alone costs
~105 us at the 360 GB/s aggregate DMA bandwidth, so everything else is
arranged to keep the DMA device saturated:
  * x and W are loaded as fp16 (half the input bytes); matmuls run
    fp16 x fp16 -> fp32 PSUM (rel err ~3e-4, far under the 2e-2 gate).
  * Each core's xT is packed host-side with its own 192 (i,b) columns
    rotated to the front, so the A path reads them straight out of the
    shared yT tile - no separate xiT input. The host un-rotates the
    j axis of the output shard with np.roll.
  * All input DMAs are emitted on the sync queue before any store, in
    dependency order: x head columns [0,256) per k slice (feeds the
    relus + A + first two B j-tiles), then W per-k slices, then the
    x tails. First stores become ready right as the input stream
    drains, so the DMA device never idles mid-kernel.

Inputs are packed host-side into SBUF layout so each is a fully
contiguous DMA (partition dim leading):
    xT   (128, KT*T) fp16:  xT[p, k*T + u] = x[(u + r*TI) % T, 128k+p]
    W_in (128, 8*D) fp16:   W_in[p, g*D+c] = W.T[128g+p, c] (g<4: W1)
    bias (1, D) fp32
    out  (12*l*b, d) fp32

Engines:
    ACT: Prelu(alpha=0.1) + B-path PSUM->SBUF drains
    PE : A/B matmuls (fp16), fp32 one-hot E-matmul for the 16->128
         partition broadcast of A[i]
    DVE: tensor_add for all output tiles + A-path PSUM drains
    DMA: big contiguous transfers; out stores are one per (i, j-group)
"""

import numpy as np
from contextlib import ExitStack

import concourse.bacc as bacc
import concourse.mybir as mybir
from concourse import tile
from concourse.bass_utils import run_bass_kernel_spmd

F32 = mybir.dt.float32
F16 = mybir.dt.float16
AF = mybir.ActivationFunctionType

L, Bdim, D = 96, 16, 512
NCORES = 8
LPC = L // NCORES          # 12 i-rows per core
T = L * Bdim               # 1536 (j,b) rows
NT = T // 128              # 12 j-tiles
KT = D // 128              # 4 k-tiles
TI = LPC * Bdim            # 192 own (i,b) rows
XH = 256                   # head columns per k slice (>= TI, 2 j-tiles)
NEG_SLOPE = 0.1


def build_nc(repeats: int = 1, group_sizes=(1, 1, 1, 1, 1, 2, 2, 3), gps_groups=()):
    """Build the per-core Bass program (identical on all cores)."""
    assert sum(group_sizes) == NT
    g_off = [0]
    for g in group_sizes:
        g_off.append(g_off[-1] + g)

    nc = bacc.Bacc("TRN2", target_bir_lowering=False, debug=False)

    xT = nc.dram_tensor("xT", (128, KT * T), F16, kind="ExternalInput")
    w_in = nc.dram_tensor("w_in", (128, 2 * KT * D), F16, kind="ExternalInput")
    bias = nc.dram_tensor("bias", (1, D), F32, kind="ExternalInput")
    out = nc.dram_tensor("out", (LPC * T, D), F32, kind="ExternalOutput")

    # One-hot E for the 16->128 partition broadcast of A rows, replicated
    # with period 32 down all 128 rows so any legal 32-aligned window has
    # identical content: ec[par][g, p] == 1 iff g % 32 == 16*par + p % 16
    ec_np = np.zeros((2, 128, 128), np.float32)
    for par in range(2):
        for g in range(128):
            for p in range(128):
                if g % 32 == 16 * par + (p % 16):
                    ec_np[par, g, p] = 1.0
    ec_dram = nc.inline_tensor(ec_np, "Ec")

    with tile.TileContext(nc) as tc, ExitStack() as ctx:
        persist = ctx.enter_context(tc.tile_pool(name="persist", bufs=1))
        stage = ctx.enter_context(tc.tile_pool(name="stage", bufs=2))
        psum = ctx.enter_context(tc.tile_pool(name="psum", bufs=6, space="PSUM"))
        outp = ctx.enter_context(tc.tile_pool(name="outp", bufs=3))

        # ---- small constants (scalar queue so they don't block sync)
        bias_sb = persist.tile([1, D], F32, tag="bias", name="bias_sb")
        nc.scalar.dma_start(bias_sb[:], bias[:])
        ones_sb = persist.tile([1, 128], F32, tag="ones", name="ones_sb")
        nc.vector.memset(ones_sb[:], 1.0)
        ec_all = persist.tile([128, 256], F32, tag="ec", name="ec_all")
        nc.scalar.dma_start(
            ec_all[:].rearrange("g (a p) -> g a p", a=2),
            ec_dram.ap().rearrange("a g p -> g a p"),
        )
        ec_sb = [ec_all[:, :128], ec_all[:, 128:]]
        w_sb = persist.tile([128, 2 * KT * D], F16, tag="w", name="w_sb")

        def w1s(k):
            return w_sb[:, k * D : (k + 1) * D]

        def w2s(k):
            return w_sb[:, (KT + k) * D : (KT + k + 1) * D]

        # ---- PE warm-up: HAM runs the PE at half clock until it has seen
        # ~3.4us of activity. Issue dummy matmuls (ones x ones) that depend
        # only on the memset so the array is at full clock when real matmuls
        # arrive with the first inputs.
        warm_ps = psum.tile([128, 64], F32, tag="eps", bufs=2, name="warm_ps")
        for _ in range(16):
            nc.tensor.matmul(
                warm_ps[:], ones_sb[:1, :], ones_sb[:1, :64],
                start=True, stop=True,
            )

        for rep in range(repeats):
            x_st = stage.tile(
                [128, KT * T], F16, tag="x_st", bufs=1, name=f"x_st_{rep}"
            )
            # ---- all input DMAs, in dependency order, before any store.
            # Head: columns [0, XH) of each k slice (own TI block + first
            # two B j-tiles) so relu/A/B can start while W still streams.
            for k in range(KT):
                nc.sync.dma_start(
                    x_st[:, k * T : k * T + XH], xT[:, k * T : k * T + XH]
                )
            if rep == 0:
                for k in range(KT):
                    nc.sync.dma_start(w_sb[:, k * D : (k + 1) * D],
                                      w_in[:, k * D : (k + 1) * D])
                for k in range(KT):
                    nc.sync.dma_start(
                        w_sb[:, (KT + k) * D : (KT + k + 1) * D],
                        w_in[:, (KT + k) * D : (KT + k + 1) * D],
                    )
            # Tails: columns [XH, T) of each k slice. Emitted before the
            # stores so the in-order sync FIFO can never park a
            # not-yet-ready store in front of pure loads.
            for k in range(KT):
                nc.sync.dma_start(
                    x_st[:, k * T + XH : (k + 1) * T],
                    xT[:, k * T + XH : (k + 1) * T],
                )

            # ---- leaky relu on the head columns (tail relus are emitted
            # after the first B drains; ACT is in-order)
            yT = persist.tile([128, KT * T], F16, tag="yT", name=f"yT_{rep}")
            for k in range(KT):
                nc.scalar.activation(
                    yT[:, k * T : k * T + XH],
                    x_st[:, k * T : k * T + XH],
                    AF.Prelu,
                    alpha=NEG_SLOPE,
                )

            # ---- A = leaky_relu(x_own) @ W1.T + bias in three M-groups
            # (128, 32, 64 rows). The own rows live in yT columns
            # [k*T, k*T + TI). The 32-row group re-covers rows 96..127 so
            # every E-matmul window starts at a legal base partition
            # (matmul operands must share base partition in {0, 32, 64}).
            a_parts = {}   # w -> (tile, offset)

            def emit_a(tag, rows, col0, windows):
                aps = psum.tile(
                    [rows, D], F32, tag="ps32", bufs=2,
                    padded_shape=[128, D], name=f"aps_{rep}_{tag}"
                )
                for k in range(KT):
                    nc.tensor.matmul(
                        aps[:],
                        yT[:, k * T + col0 : k * T + col0 + rows],
                        w1s(k),
                        start=(k == 0),
                        stop=False,
                    )
                nc.tensor.matmul(
                    aps[:], ones_sb[:1, :rows], bias_sb[:1, :],
                    start=False, stop=True,
                )
                aw = persist.tile(
                    [rows, D], F32, tag=f"a_{tag}", name=f"a_{rep}_{tag}"
                )
                nc.vector.tensor_copy(aw[:], aps[:])
                for w, off in windows:
                    a_parts[w] = (aw, off)

            out_v = out.rearrange("(i j p) c -> i p j c", i=LPC, p=128)
            abc = persist.tile([128, LPC * D], F32, tag="abc", name=f"abc_{rep}")
            n_grp = len(group_sizes)
            b_grp = [None] * n_grp

            def emit_bgroup(g):
                gsz = group_sizes[g]
                bg = persist.tile(
                    [128, gsz * D], F32, tag=f"b_grp{g}", name=f"b_grp{g}_{rep}"
                )
                for q in range(gsz):
                    jt = g_off[g] + q
                    bps = psum.tile(
                        [128, D], F32, tag="ps", bufs=4, name=f"bps_{rep}_{jt}"
                    )
                    for k in range(KT):
                        nc.tensor.matmul(
                            bps[:],
                            yT[:, k * T + 128 * jt : k * T + 128 * (jt + 1)],
                            w2s(k),
                            start=(k == 0),
                            stop=(k == KT - 1),
                        )
                    nc.scalar.activation(bg[:, q * D : (q + 1) * D], bps[:], AF.Copy)
                b_grp[g] = bg

            def emit_abc(il):
                w, par = divmod(il, 2)
                src, off = a_parts[w]
                eps = psum.tile(
                    [128, D], F32, tag="eps", bufs=2, name=f"eps_{rep}_{il}"
                )
                nc.tensor.matmul(
                    eps[:],
                    ec_sb[par][off : off + 32],
                    src[off : off + 32, :],
                    start=True,
                    stop=True,
                )
                # early slices drain on DVE (ACT is stuck behind the relus in
                # its in-order stream); later ones go to ACT which has slack.
                if il < 4:
                    nc.vector.tensor_copy(abc[:, il * D : (il + 1) * D], eps[:])
                else:
                    nc.scalar.activation(
                        abc[:, il * D : (il + 1) * D], eps[:], AF.Copy
                    )

            def emit_add(il, g):
                gsz = group_sizes[g]
                ot = outp.tile(
                    [128, gsz * D], F32, tag="ot", bufs=8,
                    name=f"ot_{rep}_{il}_{g}"
                )
                a_slice = abc[:, il * D : (il + 1) * D]
                nc.vector.tensor_add(
                    ot[:].rearrange("p (j c) -> p j c", c=D),
                    b_grp[g][:].rearrange("p (j c) -> p j c", c=D),
                    a_slice.unsqueeze(1).broadcast_to((128, gsz, D)),
                )
                nc.sync.dma_start(
                    out_v[il, :, g_off[g] : g_off[g + 1], :],
                    ot[:].rearrange("p (j c) -> p j c", c=D),
                )

            ready_il = []
            ready_g = []

            def unlock_il(*ils):
                for il in ils:
                    emit_abc(il)
                for il in ils:
                    ready_il.append(il)
                    for g in ready_g:
                        emit_add(il, g)

            def unlock_g(g):
                emit_bgroup(g)
                ready_g.append(g)
                for il in ready_il:
                    emit_add(il, g)

            def emit_relu_tail():
                for k in range(KT):
                    nc.scalar.activation(
                        yT[:, k * T + XH : (k + 1) * T],
                        x_st[:, k * T + XH : (k + 1) * T],
                        AF.Prelu,
                        alpha=NEG_SLOPE,
                    )

            # windows: w0..2 live in the 128-row A group at offsets 0/32/64,
            # w3 in its own 32-row group, w4..5 in the 64-row group.
            a_specs = {
                0: ("g0", 128, 0, [(0, 0), (1, 32), (2, 64)]),
                3: ("g0b", 32, 96, [(3, 0)]),
                4: ("g1", 64, 128, [(4, 0), (5, 32)]),
            }
            # Head: A chain, then the first two B j-tile groups (their
            # columns are already in the head slice), then the first abc
            # broadcasts + adds. PE order: A, B g0, B g1, E0, E1 keeps the
            # B drains ahead of the aw-drain stall.
            emit_a(*a_specs[0])
            emit_bgroup(0)
            ready_g.append(0)
            emit_bgroup(1)
            ready_g.append(1)
            unlock_il(0, 1)
            emit_relu_tail()

            pairs = [(2 * p, 2 * p + 1) for p in range(LPC // 2)]
            gi = 2
            for pi, pair in enumerate(pairs):
                if pi == 0:
                    continue
                if pi in a_specs:
                    emit_a(*a_specs[pi])
                unlock_il(*pair)
                while gi < n_grp and (gi + 1) * len(pairs) <= (pi + 1) * n_grp:
                    unlock_g(gi)
                    gi += 1
            while gi < n_grp:
                unlock_g(gi)
                gi += 1

    nc.compile()
    return nc


def _pack_kt(arr_t, nfree):
    """(D, nfree) k-major -> (128, KT*nfree) partition-packed SBUF layout."""
    return np.ascontiguousarray(
        arr_t.reshape(KT, 128, nfree).transpose(1, 0, 2).reshape(128, KT * nfree)
    )


def make_in_maps(x, W, bias):
    x = np.asarray(x, np.float32)
    W = np.asarray(W, np.float32)
    bias = np.asarray(bias, np.float32)
    x_t = np.ascontiguousarray(x.reshape(T, D).astype(np.float16))
    w_all = np.ascontiguousarray(
        np.ascontiguousarray(W.T.astype(np.float16))
        .reshape(2 * KT, 128, D)
        .transpose(1, 0, 2)
        .reshape(128, 2 * KT * D)
    )
    b2 = np.ascontiguousarray(bias.reshape(1, D))
    maps = []
    for r in range(NCORES):
        # rotate this core's own TI rows to the front of the t axis
        x_rot = np.roll(x_t, -r * TI, axis=0)
        xTr = _pack_kt(np.ascontiguousarray(x_rot.T), T)
        maps.append({"xT": xTr, "w_in": w_all, "bias": b2})
    return maps


_NC_CACHE = {}


def get_nc(repeats=1, group_sizes=(1, 1, 1, 1, 1, 2, 2, 3), gps_groups=()):
    key = (repeats, tuple(group_sizes), tuple(gps_groups))
    if key not in _NC_CACHE:
        _NC_CACHE[key] = build_nc(repeats=repeats, group_sizes=group_sizes, gps_groups=gps_groups)
    return _NC_CACHE[key]


def kernel(x, W, bias, group_sizes=(1, 1, 1, 1, 1, 2, 2, 3), gps_groups=()):
    nc = get_nc(1, group_sizes, gps_groups)
    maps = make_in_maps(x, W, bias)
    res = run_bass_kernel_spmd(nc, maps, list(range(NCORES)))
    outs = []
    for r in range(NCORES):
        o = res.results[r]["out"].reshape(LPC, T, D)
        outs.append(np.roll(o, r * TI, axis=1).reshape(LPC * T, D))
    return np.concatenate(outs, axis=0).reshape(L * L, Bdim, D)


# revision 10
# speedup vs baseline: 1.0663x; 1.0259x over previous
"""CatLayer Trainium2 kernel (fp16 inputs, fp32 accumulate/output).

Math: out[i,j,b,:] = W @ leaky_relu(concat(x[i,b,:], x[j,b,:])) + bias
Since leaky_relu is elementwise over the concat:
    y  = leaky_relu(x)                    # (l, b, d)
    A  = y @ W[:, :d].T + bias            # (l, b, d)   "xi half"
    B  = y @ W[:, d:].T                   # (l, b, d)   "xj half"
    out[i,j,b,:] = A[i,b,:] + B[j,b,:]

Sharding: i-rows of the (l x l) pair grid over 8 cores (12 rows each).
Every core computes B for all j from full x; A only for its own i rows.

The kernel is DMA-bound: the fp32 output (37.75 MB/core) alone costs
~105 us at the 360 GB/s aggregate DMA bandwidth, so everything else is
arranged to keep the DMA device saturated:
  * x and W are loaded as fp16 (half the input bytes); matmuls run
    fp16 x fp16 -> fp32 PSUM (rel err ~3e-4, far under the 2e-2 gate).
  * Each core's xT is packed host-side with its own 192 (i,b) columns
    rotated to the front, so the A path reads them straight out of the
    shared yT tile - no separate xiT input. The host un-rotates the
    j axis of the output shard with np.roll.
  * All input DMAs are emitted on the sync queue before any store, in
    dependency order: x head columns [0,256) per k slice (feeds the
    relus + A + first two B j-tiles), then W per-k slices, then the
    x tails. First stores become ready right as the input stream
    drains, so the DMA device never idles mid-kernel.

Inputs are packed host-side into SBUF layout so each is a fully
contiguous DMA (partition dim leading):
    xT   (128, KT*T) fp16:  xT[p, k*T + u] = x[(u + r*TI) % T, 128k+p]
    W_in (128, 8*D) fp16:   W_in[p, g*D+c] = W.T[128g+p, c] (g<4: W1)
    bias (1, D) fp32
    out  (12*l*b, d) fp32

Engines:
    ACT: Prelu(alpha=0.1) + B-path PSUM->SBUF drains
    PE : A/B matmuls (fp16), fp32 one-hot E-matmul for the 16->128
         partition broadcast of A[i]
    DVE: tensor_add for all output tiles + A-path PSUM drains
    DMA: big contiguous transfers; out stores are one per (i, j-group)
"""

import numpy as np
from contextlib import ExitStack

import concourse.bacc as bacc
import concourse.mybir as mybir
from concourse import tile
from concourse.bass_utils import run_bass_kernel_spmd

F32 = mybir.dt.float32
F16 = mybir.dt.float16
AF = mybir.ActivationFunctionType

L, Bdim, D = 96, 16, 512
NCORES = 8
LPC = L // NCORES          # 12 i-rows per core
T = L * Bdim               # 1536 (j,b) rows
NT = T // 128              # 12 j-tiles
KT = D // 128              # 4 k-tiles
TI = LPC * Bdim            # 192 own (i,b) rows
XH = 256                   # head columns per k slice (>= TI, 2 j-tiles)
NEG_SLOPE = 0.1


def build_nc(repeats: int = 1, group_sizes=(1, 1, 1, 1, 1, 2, 2, 3), gps_groups=()):
    """Build the per-core Bass program (identical on all cores)."""
    assert sum(group_sizes) == NT
    g_off = [0]
    for g in group_sizes:
        g_off.append(g_off[-1] + g)

    nc = bacc.Bacc("TRN2", target_bir_lowering=False, debug=False)

    xT = nc.dram_tensor("xT", (128, KT * T), F16, kind="ExternalInput")
    w_in = nc.dram_tensor("w_in", (128, 2 * KT * D), F16, kind="ExternalInput")
    bias = nc.dram_tensor("bias", (1, D), F32, kind="ExternalInput")
    out = nc.dram_tensor("out", (LPC * T, D), F32, kind="ExternalOutput")

    # One-hot E for the 16->128 partition broadcast of A rows, replicated
    # with period 32 down all 128 rows so any legal 32-aligned window has
    # identical content: ec[par][g, p] == 1 iff g % 32 == 16*par + p % 16
    ec_np = np.zeros((2, 128, 128), np.float16)
    for par in range(2):
        for g in range(128):
            for p in range(128):
                if g % 32 == 16 * par + (p % 16):
                    ec_np[par, g, p] = 1.0
    ec_dram = nc.inline_tensor(ec_np, "Ec")

    with tile.TileContext(nc) as tc, ExitStack() as ctx:
        persist = ctx.enter_context(tc.tile_pool(name="persist", bufs=1))
        stage = ctx.enter_context(tc.tile_pool(name="stage", bufs=2))
        psum = ctx.enter_context(tc.tile_pool(name="psum", bufs=6, space="PSUM"))
        outp = ctx.enter_context(tc.tile_pool(name="outp", bufs=3))

        # ---- small constants (scalar queue so they don't block sync)
        bias_sb = persist.tile([1, D], F32, tag="bias", name="bias_sb")
        nc.scalar.dma_start(bias_sb[:], bias[:])
        ones_sb = persist.tile([1, 128], F32, tag="ones", name="ones_sb")
        nc.vector.memset(ones_sb[:], 1.0)
        ec_all = persist.tile([128, 256], F16, tag="ec", name="ec_all")
        nc.scalar.dma_start(
            ec_all[:].rearrange("g (a p) -> g a p", a=2),
            ec_dram.ap().rearrange("a g p -> g a p"),
        )
        ec_sb = [ec_all[:, :128], ec_all[:, 128:]]
        w_sb = persist.tile([128, 2 * KT * D], F16, tag="w", name="w_sb")

        def w1s(k):
            return w_sb[:, k * D : (k + 1) * D]

        def w2s(k):
            return w_sb[:, (KT + k) * D : (KT + k + 1) * D]

        # ---- PE warm-up: HAM runs the PE at half clock until it has seen
        # ~3.4us of activity. Issue dummy matmuls (ones x ones) wide enough
        # to keep the array busy until the first real matmuls arrive with
        # W1 (~4.2us), so they run at full clock.
        warm_ps = psum.tile([128, D], F32, tag="eps", bufs=2, name="warm_ps")
        for _ in range(10):
            nc.tensor.matmul(
                warm_ps[:, :128], ones_sb[:1, :], ones_sb[:1, :],
                start=True, stop=True,
            )

        for rep in range(repeats):
            x_st = stage.tile(
                [128, KT * T], F16, tag="x_st", bufs=1, name=f"x_st_{rep}"
            )
            # ---- all input DMAs, in dependency order, before any store.
            # Four big transfers: each DMA has a ~650ns descriptor-gen pitch
            # through SEQ/HWDGE, so small per-k slices would starve the DMA
            # engines. Strided APs batch all k slices into one instruction.
            # Head: columns [0, XH) of each k slice (own TI block + first
            # two B j-tiles) so relu/A/B can start while W still streams.
            x_st_v = x_st[:].rearrange("p (k t) -> p k t", k=KT)
            xT_v = xT.ap().rearrange("p (k t) -> p k t", k=KT)
            nc.sync.dma_start(x_st_v[:, :, :XH], xT_v[:, :, :XH])
            if rep == 0:
                nc.sync.dma_start(w_sb[:, : KT * D], w_in[:, : KT * D])
                nc.sync.dma_start(w_sb[:, KT * D :], w_in[:, KT * D :])
            # Tails: columns [XH, T) of each k slice. Emitted before the
            # stores so the in-order sync FIFO can never park a
            # not-yet-ready store in front of pure loads.
            nc.sync.dma_start(x_st_v[:, :, XH:], xT_v[:, :, XH:])

            # ---- leaky relu on the head columns (tail relus are emitted
            # after the first B drains; ACT is in-order)
            yT = persist.tile([128, KT * T], F16, tag="yT", name=f"yT_{rep}")
            for k in range(KT):
                nc.scalar.activation(
                    yT[:, k * T : k * T + XH],
                    x_st[:, k * T : k * T + XH],
                    AF.Prelu,
                    alpha=NEG_SLOPE,
                )

            # ---- A = leaky_relu(x_own) @ W1.T + bias in three M-groups
            # (128, 32, 64 rows). The own rows live in yT columns
            # [k*T, k*T + TI). The 32-row group re-covers rows 96..127 so
            # every E-matmul window starts at a legal base partition
            # (matmul operands must share base partition in {0, 32, 64}).
            a_parts = {}   # w -> (tile, offset)

            def emit_a(tag, rows, col0, windows):
                aps = psum.tile(
                    [rows, D], F32, tag="ps32", bufs=2,
                    padded_shape=[128, D], name=f"aps_{rep}_{tag}"
                )
                for k in range(KT):
                    nc.tensor.matmul(
                        aps[:],
                        yT[:, k * T + col0 : k * T + col0 + rows],
                        w1s(k),
                        start=(k == 0),
                        stop=False,
                    )
                nc.tensor.matmul(
                    aps[:], ones_sb[:1, :rows], bias_sb[:1, :],
                    start=False, stop=True,
                )
                # drain to fp16 so the E-matmul broadcast runs at 1 cyc/row
                aw = persist.tile(
                    [rows, D], F16, tag=f"a_{tag}", name=f"a_{rep}_{tag}"
                )
                nc.vector.tensor_copy(aw[:], aps[:])
                for w, off in windows:
                    a_parts[w] = (aw, off)

            out_v = out.rearrange("(i j p) c -> i p j c", i=LPC, p=128)
            abc = persist.tile([128, LPC * D], F32, tag="abc", name=f"abc_{rep}")
            n_grp = len(group_sizes)
            b_grp = [None] * n_grp

            def emit_bgroup(g):
                gsz = group_sizes[g]
                bg = persist.tile(
                    [128, gsz * D], F32, tag=f"b_grp{g}", name=f"b_grp{g}_{rep}"
                )
                for q in range(gsz):
                    jt = g_off[g] + q
                    bps = psum.tile(
                        [128, D], F32, tag="ps", bufs=4, name=f"bps_{rep}_{jt}"
                    )
                    for k in range(KT):
                        nc.tensor.matmul(
                            bps[:],
                            yT[:, k * T + 128 * jt : k * T + 128 * (jt + 1)],
                            w2s(k),
                            start=(k == 0),
                            stop=(k == KT - 1),
                        )
                    nc.scalar.activation(bg[:, q * D : (q + 1) * D], bps[:], AF.Copy)
                b_grp[g] = bg

            def emit_abc(il):
                w, par = divmod(il, 2)
                src, off = a_parts[w]
                eps = psum.tile(
                    [128, D], F32, tag="eps", bufs=2, name=f"eps_{rep}_{il}"
                )
                nc.tensor.matmul(
                    eps[:],
                    ec_sb[par][off : off + 32],
                    src[off : off + 32, :],
                    start=True,
                    stop=True,
                )
                # early slices drain on DVE (ACT is stuck behind the relus in
                # its in-order stream); later ones go to ACT which has slack.
                if il < 4:
                    nc.vector.tensor_copy(abc[:, il * D : (il + 1) * D], eps[:])
                else:
                    nc.scalar.activation(
                        abc[:, il * D : (il + 1) * D], eps[:], AF.Copy
                    )

            def emit_add(il, g):
                gsz = group_sizes[g]
                ot = outp.tile(
                    [128, gsz * D], F32, tag="ot", bufs=8,
                    name=f"ot_{rep}_{il}_{g}"
                )
                a_slice = abc[:, il * D : (il + 1) * D]
                nc.vector.tensor_add(
                    ot[:].rearrange("p (j c) -> p j c", c=D),
                    b_grp[g][:].rearrange("p (j c) -> p j c", c=D),
                    a_slice.unsqueeze(1).broadcast_to((128, gsz, D)),
                )
                nc.sync.dma_start(
                    out_v[il, :, g_off[g] : g_off[g + 1], :],
                    ot[:].rearrange("p (j c) -> p j c", c=D),
                )

            ready_il = []
            ready_g = []

            def unlock_il(*ils):
                for il in ils:
                    emit_abc(il)
                for il in ils:
                    ready_il.append(il)
                    for g in ready_g:
                        emit_add(il, g)

            def unlock_g(g):
                emit_bgroup(g)
                ready_g.append(g)
                for il in ready_il:
                    emit_add(il, g)

            def emit_relu_tail(c0, c1):
                for k in range(KT):
                    nc.scalar.activation(
                        yT[:, k * T + c0 : k * T + c1],
                        x_st[:, k * T + c0 : k * T + c1],
                        AF.Prelu,
                        alpha=NEG_SLOPE,
                    )

            # windows: w0..2 live in the 128-row A group at offsets 0/32/64,
            # w3 in its own 32-row group, w4..5 in the 64-row group.
            a_specs = {
                0: ("g0", 128, 0, [(0, 0), (1, 32), (2, 64)]),
                3: ("g0b", 32, 96, [(3, 0)]),
                4: ("g1", 64, 128, [(4, 0), (5, 32)]),
            }
            # Head: A chain, then the first two B j-tile groups (their
            # columns are already in the head slice), then the first abc
            # broadcasts + adds. PE order: A, B g0, B g1, E0, E1 keeps the
            # B drains ahead of the aw-drain stall. The tail relus are
            # split: j-tiles 2..3 right after the first drains so B g2/g3
            # aren't parked behind the full 4.3us relu stream on the
            # in-order ACT queue.
            emit_a(*a_specs[0])
            emit_bgroup(0)
            ready_g.append(0)
            emit_bgroup(1)
            ready_g.append(1)
            unlock_il(0, 1)
            emit_relu_tail(XH, XH + 256)

            pairs = [(2 * p, 2 * p + 1) for p in range(LPC // 2)]
            gi = 2
            for pi, pair in enumerate(pairs):
                if pi == 0:
                    continue
                if pi in a_specs:
                    emit_a(*a_specs[pi])
                unlock_il(*pair)
                if pi == 1:
                    unlock_g(2)
                    gi = 3
                    emit_relu_tail(XH + 256, T)
                else:
                    while gi < n_grp and (gi + 1) * len(pairs) <= (pi + 1) * n_grp:
                        unlock_g(gi)
                        gi += 1
            while gi < n_grp:
                unlock_g(gi)
                gi += 1

    nc.compile()
    return nc


def _pack_kt(arr_t, nfree):
    """(D, nfree) k-major -> (128, KT*nfree) partition-packed SBUF layout."""
    return np.ascontiguousarray(
        arr_t.reshape(KT, 128, nfree).transpose(1, 0, 2).reshape(128, KT * nfree)
    )


def make_in_maps(x, W, bias):
    x = np.asarray(x, np.float32)
    W = np.asarray(W, np.float32)
    bias = np.asarray(bias, np.float32)
    x_t = np.ascontiguousarray(x.reshape(T, D).astype(np.float16))
    w_all = np.ascontiguousarray(
        np.ascontiguousarray(W.T.astype(np.float16))
        .reshape(2 * KT, 128, D)
        .transpose(1, 0, 2)
        .reshape(128, 2 * KT * D)
    )
    b2 = np.ascontiguousarray(bias.reshape(1, D))
    maps = []
    for r in range(NCORES):
        # rotate this core's own TI rows to the front of the t axis
        x_rot = np.roll(x_t, -r * TI, axis=0)
        xTr = _pack_kt(np.ascontiguousarray(x_rot.T), T)
        maps.append({"xT": xTr, "w_in": w_all, "bias": b2})
    return maps


_NC_CACHE = {}


def get_nc(repeats=1, group_sizes=(1, 1, 1, 1, 1, 2, 2, 3), gps_groups=()):
    key = (repeats, tuple(group_sizes), tuple(gps_groups))
    if key not in _NC_CACHE:
        _NC_CACHE[key] = build_nc(repeats=repeats, group_sizes=group_sizes, gps_groups=gps_groups)
    return _NC_CACHE[key]


def kernel(x, W, bias, group_sizes=(1, 1, 1, 1, 1, 2, 2, 3), gps_groups=()):
    nc = get_nc(1, group_sizes, gps_groups)
    maps = make_in_maps(x, W, bias)
    res = run_bass_kernel_spmd(nc, maps, list(range(NCORES)))
    outs = []
    for r in range(NCORES):
        o = res.results[r]["out"].reshape(LPC, T, D)
        outs.append(np.roll(o, r * TI, axis=1).reshape(LPC * T, D))
    return np.concatenate(outs, axis=0).reshape(L * L, Bdim, D)
